# revision 24
# baseline (speedup 1.0000x reference)
"""AgentNet GNN message-passing kernel for 8 Trainium2 NeuronCores.

Algebraic collapse (validated to 4e-7 rel err vs reference in fp32):
  hidden = relu(obs @ enc_w + enc_b)                      [N,128]
  q'     = hidden @ Wq2 + bq2      (q/k projections folded; key bias
                                    cancels in softmax)   [N,128]
  s[n,k] = q'[n] . hidden[idx[n,k]]
  attn   = softmax_k(s)            (scores are O(0.05) -> no max-subtract)
  y      = sum_k attn[n,k] * (hidden[idx[n,k]] @ W2)      [N,32]
  logits = hidden @ dec_w + y + b2
  out    = softmax_a(logits)

The neighbor gather commutes with the row-wise encoder, so the host gathers
obs rows per edge (cheap numpy take) and the device runs a dense per-edge
encoder matmul -- no data-dependent DMA on device.

v2 layout: the edge encoder emits TRANSPOSED hiddens gT[c,(k,p)] per
supertile (PE cost identical -- swap lhsT/rhs).  That moves both
reductions onto the idle PE:
  - scores: prod2[c,(k,p)] = gT * qT (DVE TT @2x), then 16 accumulating
    one-hot matmuls reduce over c -> sT[k,p] in PSUM; one PE transpose
    brings s back to node orientation for the softmax.
  - value: zg[p,(k,a)] = gT_k^T @ W2 on PE (a=32 wide, 4x smaller than
    hidden), then one DVE TT applies attn and a short k-tree finishes.
Engine knobs below split relu / tree / small ops across Act/DVE/Pool.

Sharding: nodes split into 8 contiguous shards (6250/core, padded to 6656).
Weights replicated; no collectives.
"""
import numpy as np

N, K = 50000, 16
IN_DIM, H, C, A = 64, 128, 128, 32
NCORES = 8
SHARD = N // NCORES              # 6250
P = 128
NSUP = 52                        # supertiles of 128 nodes per core (6656)
NS = NSUP * P                    # 6656 padded shard nodes
CSUP = 4                         # supertiles per chunk
NCHUNK = NSUP // CSUP            # 13
EPC = CSUP * P * K               # 8192 edges per chunk

# ---- engine assignment (tuned against the TimelineSim cost model) ----
RELU_ENG = ("act", "act", "act", "act", "act", "act", "act", "dve",
            "act", "act", "act", "dve", "act", "act", "act", "act")
STAGEB_HID_ENG = "act"           # relu of hidT block
TREE_ENG = ("pool", "pool", "pool", "pool")   # k-tree levels 8,4,2,1
PR_ENG = "pool"                  # final prob scale
YS_ENG = "pool"                  # 1/Z value scale

_PROG_CACHE = {}


def _build_program():
    import concourse.bacc as bacc
    import concourse.mybir as mybir
    import concourse.tile as tile
    from concourse.masks import make_identity

    f32 = mybir.dt.float32
    bf16 = mybir.dt.bfloat16
    AX = mybir.AxisListType
    OP = mybir.AluOpType
    AF = mybir.ActivationFunctionType

    nc = bacc.Bacc("TRN2", target_bir_lowering=False, debug=False,
                   num_devices=1)

    # per-edge pre-gathered obs, transposed, with a trailing ones-row for the
    # encoder bias fold; column order: chunk j | supertile g | k | partition p
    obsgT = nc.dram_tensor("obsgT", [IN_DIM + 1, NS * K], bf16, kind="ExternalInput")
    obsT_shard = nc.dram_tensor("obsT_shard", [IN_DIM + 1, NS], bf16, kind="ExternalInput")
    # all bf16 weights packed into one tensor (single DMA at startup):
    # cols 0:128 enc_wb (rows 0:65) | 128:256 wq2 | 256:288 dec_w
    # | 288:320 w2 | 320:352 b2row (row 0)
    wpack = nc.dram_tensor("wpack", [P, 352], bf16, kind="ExternalInput")
    bq2col = nc.dram_tensor("bq2col", [C, 1], f32, kind="ExternalInput")
    outp = nc.dram_tensor("outp", [P, NSUP * A], bf16, kind="ExternalOutput")

    with tile.TileContext(nc) as tc:
        with tc.tile_pool(name="cst", bufs=1) as cst, \
             tc.tile_pool(name="obs", bufs=3) as obsp, \
             tc.tile_pool(name="big", bufs=1) as big, \
             tc.tile_pool(name="gts", bufs=4) as gts, \
             tc.tile_pool(name="pw", bufs=4) as pwp, \
             tc.tile_pool(name="sml", bufs=6) as sml, \
             tc.tile_pool(name="psG", bufs=2, space="PSUM") as psG, \
             tc.tile_pool(name="psS", bufs=2, space="PSUM") as psS, \
             tc.tile_pool(name="psZ", bufs=1, space="PSUM") as psZ, \
             tc.tile_pool(name="psL", bufs=1, space="PSUM") as psL:

            # ---- constants (one packed DMA for all bf16 weights) ----
            wpk = cst.tile([P, 352], bf16)
            nc.sync.dma_start(out=wpk[:], in_=wpack[:, :])
            encwb_sb = wpk[:IN_DIM + 1, 0:H]
            wq2_sb = wpk[:, H:2 * H]
            decw_sb = wpk[:, 256:256 + A]
            w2_sb = wpk[:, 288:288 + A]
            b2_sb = wpk[:1, 320:320 + A]
            bq2_sb = cst.tile([C, 1], f32)
            nc.sync.dma_start(out=bq2_sb[:], in_=bq2col[:, :])
            ones1 = cst.tile([1, P], bf16)
            nc.vector.memset(ones1[:], 1.0)
            onesPK = cst.tile([P, K], bf16)
            nc.vector.memset(onesPK[:], 1.0)
            ident = cst.tile([P, P], bf16)
            make_identity(nc, ident[:])
            identf = cst.tile([K, K], f32)
            make_identity(nc, identf[:])
            # onehot[:, k*16+j] = 1 iff j == k: lhsT for the per-k score
            # reduction (sum over c lands in psum partition k only)
            onehot = cst.tile([P, K * K], bf16)
            nc.vector.memset(onehot[:], 0.0)
            for k in range(K):
                nc.vector.memset(onehot[:, k * K + k:k * K + k + 1], 1.0)

            # persistent: transposed hidden + transposed q' for the shard
            hidT = big.tile([H, NS], bf16)
            qT = big.tile([C, NS], bf16)
            # decode-logit psum: ping-pong halves across chunks; the tail
            # 128 cols hold 8 rotating esc[p,16] slots (keeps the score-psum
            # tile's lifetime short -- it frees right after the exp)
            plbig = psL.tile([P, 2 * CSUP * A], f32, tag="pl",
                             space="PSUM")

            def open_chunk(j):
                nb = j * 512
                # stage B block j: hidT, qT for nodes nb..nb+512
                obsb = obsp.tile([IN_DIM + 1, 512], bf16, tag="ob")
                nc.sync.dma_start(out=obsb[:], in_=obsT_shard[:, nb:nb + 512])
                phb = psS.tile([H, 512], f32, tag="sc", space="PSUM")
                ph = phb[:, :512]
                nc.tensor.matmul(out=ph, lhsT=encwb_sb, rhs=obsb[:],
                                 start=True, stop=True)
                if STAGEB_HID_ENG == "act":
                    nc.scalar.activation(out=hidT[:, nb:nb + 512], in_=ph,
                                         func=AF.Relu)
                else:
                    nc.vector.tensor_scalar_max(out=hidT[:, nb:nb + 512],
                                                in0=ph, scalar1=0.0)
                pqb = psS.tile([C, 512], f32, tag="sc", space="PSUM")
                pq = pqb[:, :512]
                nc.tensor.matmul(out=pq, lhsT=wq2_sb,
                                 rhs=hidT[:, nb:nb + 512],
                                 start=True, stop=True)
                # copy + query bias in one Act pass (bias is per-partition)
                nc.scalar.activation(out=qT[:, nb:nb + 512], in_=pq,
                                     func=AF.Identity, bias=bq2_sb[:, :])

                og = obsp.tile([IN_DIM + 1, EPC], bf16, tag="og")
                for q4 in range(CSUP):
                    nc.sync.dma_start(
                        out=og[:, q4 * 2048:(q4 + 1) * 2048],
                        in_=obsgT[:, j * EPC + q4 * 2048:
                                  j * EPC + (q4 + 1) * 2048])
                ychunk = sml.tile([P, CSUP * A], bf16, tag="y", name=f"y{j}")
                zC = sml.tile([P, CSUP], f32, tag="zC", name=f"zC{j}")
                rzC = sml.tile([P, CSUP], f32, tag="rzC", name=f"rzC{j}")
                return {
                    "og": og,
                    "pl": plbig[:, (j % 2) * CSUP * A:(j % 2 + 1) * CSUP * A],
                    "y": ychunk,
                    "zC": zC,
                    "rzC": rzC,
                }

            def emit_edge(s, ctx):
                # edge encoder, transposed output: gT[c, (k,p)]
                gi = s % CSUP
                og = ctx["og"]
                gt = gts.tile([C, K * P], bf16, tag="g")
                for h in range(2):
                    gps = psG.tile([C, 1024], f32, tag="gps", space="PSUM")
                    for q in range(2):
                        nc.tensor.matmul(
                            out=gps[:, q * 512:(q + 1) * 512],
                            lhsT=encwb_sb,
                            rhs=og[:, gi * 2048 + h * 1024 + q * 512:
                                   gi * 2048 + h * 1024 + (q + 1) * 512],
                            start=True, stop=True)
                    dst = gt[:, h * 1024:(h + 1) * 1024]
                    if RELU_ENG[(s * 2 + h) % len(RELU_ENG)] == "act":
                        nc.scalar.activation(out=dst, in_=gps[:], func=AF.Relu)
                    else:
                        nc.vector.tensor_scalar_max(out=dst, in0=gps[:],
                                                    scalar1=0.0)
                return gt

            def emit_value(s, ctx, gt):
                j, gi = divmod(s, CSUP)
                nb = j * 512
                zC, rzC, ychunk, pl = ctx["zC"], ctx["rzC"], ctx["y"], ctx["pl"]
                # score products: prod2[c, k, p] = gT * qT (bcast over k),
                # split in k-halves so the PE reduce starts early
                prod2 = pwp.tile([C, K * P], bf16, tag="pw")
                p3 = prod2[:].rearrange("c (k p) -> c k p", p=P)
                g3 = gt[:].rearrange("c (k p) -> c k p", p=P)
                qb = qT[:, None, nb + gi * P:nb + (gi + 1) * P].to_broadcast(
                    [C, K, P])
                nc.vector.tensor_tensor(
                    out=p3[:, :8, :], in0=g3[:, :8, :],
                    in1=qb[:, :8, :], op=OP.mult)
                nc.vector.tensor_tensor(
                    out=p3[:, 8:, :], in0=g3[:, 8:, :],
                    in1=qb[:, 8:, :], op=OP.mult)

                # score reduce over c on PE: sT[k, p] via one-hot lhsT
                scps = psS.tile([P, 512], f32, tag="sc", space="PSUM")
                sT = scps[:K, :P]
                for k in range(K):
                    nc.tensor.matmul(
                        out=sT, lhsT=onehot[:, k * K:(k + 1) * K],
                        rhs=prod2[:, k * P:(k + 1) * P],
                        start=(k == 0), stop=(k == K - 1))
                # exp in k-orientation (fuses the psum drain with exp),
                # then transpose the exp'd scores to node orientation
                escT = sml.tile([K, P], f32, tag="sTsb")
                nc.scalar.activation(out=escT[:], in_=sT, func=AF.Exp)
                esc_ps = scps[:, P:P + K]
                nc.tensor.transpose(out=esc_ps, in_=escT[:],
                                    identity=identf[:K, :K])
                # drain esc PSUM->SBUF (HW allows only one PSUM operand per
                # TensorTensor and wp needs zg from PSUM), then the softmax
                # denominator off the critical path
                esc = sml.tile([P, K], bf16, tag="esc")
                nc.vector.tensor_scalar_mul(out=esc[:], in0=esc_ps,
                                            scalar1=1.0)
                nc.vector.reduce_sum(
                    out=zC[:, gi:gi + 1], in_=esc[:, None, :], axis=AX.X)
                nc.vector.reciprocal(out=rzC[:, gi:gi + 1],
                                     in_=zC[:, gi:gi + 1])

                # value path: zg[p, k, a] = gT_k^T @ W2 on PE
                zg = psZ.tile([P, K * A], f32, tag="zg", space="PSUM")
                for k in range(K):
                    nc.tensor.matmul(
                        out=zg[:, k * A:(k + 1) * A],
                        lhsT=gt[:, k * P:(k + 1) * P],
                        rhs=w2_sb, start=True, stop=True)
                wp = pwp.tile([P, K * A], bf16, tag="wp")
                w3 = wp[:].rearrange("p (k a) -> p k a", a=A)
                nc.vector.tensor_tensor(
                    out=w3[:, :, :],
                    in0=zg[:].rearrange("p (k a) -> p k a", a=A),
                    in1=esc[:, :, None].to_broadcast([P, K, A]),
                    op=OP.mult)
                # k-tree: 16 -> 8 -> 4 -> 2 -> 1, then 1/Z scale
                for li, w in enumerate((8, 4, 2)):
                    eng = nc.gpsimd if TREE_ENG[li] == "pool" else nc.vector
                    eng.tensor_tensor(
                        out=w3[:, :w, :], in0=w3[:, :w, :],
                        in1=w3[:, w:2 * w, :], op=OP.add)
                eng = nc.gpsimd if TREE_ENG[3] == "pool" else nc.vector
                eng.tensor_tensor(
                    out=w3[:, 0:1, :],
                    in0=w3[:, 0:1, :], in1=w3[:, 1:2, :], op=OP.add)
                yeng = nc.gpsimd if YS_ENG == "pool" else nc.vector
                yeng.tensor_scalar_mul(
                    out=ychunk[:, gi * A:(gi + 1) * A], in0=wp[:, :A],
                    scalar1=rzC[:, gi:gi + 1])

                # decode logits: hidden @ dec_w + b2 (y added below)
                sl = pl[:, gi * A:(gi + 1) * A]
                nc.tensor.matmul(out=sl, lhsT=hidT[:, s * P:(s + 1) * P],
                                 rhs=decw_sb, start=True, stop=False)
                nc.tensor.matmul(out=sl, lhsT=ones1[:], rhs=b2_sb,
                                 start=False, stop=True)

            def emit_tail(j, ctx):
                # decode softmax for the chunk
                eli = sml.tile([P, CSUP * A], f32, tag="eli")
                nc.vector.tensor_tensor(out=eli[:], in0=ctx["pl"],
                                        in1=ctx["y"][:], op=OP.add)
                el = sml.tile([P, CSUP * A], bf16, tag="el")
                nc.scalar.activation(out=el[:], in_=eli[:], func=AF.Exp)
                zl = sml.tile([P, CSUP], f32, tag="zl")
                nc.vector.reduce_sum(
                    out=zl[:], in_=el[:].rearrange("p (s a) -> p s a", a=A),
                    axis=AX.X)
                rzl = sml.tile([P, CSUP], f32, tag="rzl")
                nc.vector.reciprocal(out=rzl[:], in_=zl[:])
                pr = sml.tile([P, CSUP * A], bf16, tag="pr")
                eng = nc.gpsimd if PR_ENG == "pool" else nc.vector
                eng.tensor_tensor(
                    out=pr[:].rearrange("p (s a) -> p s a", a=A),
                    in0=el[:].rearrange("p (s a) -> p s a", a=A),
                    in1=rzl[:, :, None].to_broadcast([P, CSUP, A]),
                    op=OP.mult)
                nc.sync.dma_start(
                    out=outp[:, j * CSUP * A:(j + 1) * CSUP * A], in_=pr[:])

            # software-pipelined: edge(s) runs one supertile ahead of the
            # attention/value work so the PE never head-of-line blocks on the
            # single zg psum bank
            prev = None
            for s in range(NSUP):
                j, gi = divmod(s, CSUP)
                if gi == 0:
                    ctx = open_chunk(j)
                gt = emit_edge(s, ctx)
                if prev is not None:
                    emit_value(*prev)
                    ps = prev[0]
                    if ps % CSUP == CSUP - 1:
                        emit_tail(ps // CSUP, prev[1])
                prev = (s, ctx, gt)
            emit_value(*prev)
            emit_tail(NCHUNK - 1, prev[1])

    nc.compile()
    return nc


def _fold_weights(enc_w, enc_b, msg_w, msg_b, key_w, key_b,
                  in_proj_w, in_proj_b, out_w, out_b, dec_w, dec_b):
    wq, wk, wv = in_proj_w[:C], in_proj_w[C:2 * C], in_proj_w[2 * C:]
    bq, bv = in_proj_b[:C], in_proj_b[2 * C:]
    Wq_eff = msg_w @ wq.T
    bq_eff = msg_b @ wq.T + bq
    Wk_eff = key_w @ wk.T
    Wv_eff = msg_w @ wv.T
    bv_eff = msg_b @ wv.T + bv
    s = np.float32(1.0 / np.sqrt(np.float32(C)))
    Wq2 = (Wq_eff @ Wk_eff.T) * s
    bq2 = (bq_eff @ Wk_eff.T) * s
    W2 = Wv_eff @ out_w @ dec_w
    b2 = bv_eff @ out_w @ dec_w + out_b @ dec_w + dec_b
    enc_w65 = np.concatenate([enc_w, enc_b[None, :]], axis=0)
    return enc_w65.astype(np.float32), Wq2.astype(np.float32), \
        bq2.astype(np.float32), W2.astype(np.float32), b2.astype(np.float32)


def _prep_in_maps(obs, neighbor_idx, enc_w, enc_b, msg_w, msg_b, key_w,
                  key_b, in_proj_w, in_proj_b, out_w, out_b, dec_w, dec_b):
    import ml_dtypes

    bf = ml_dtypes.bfloat16
    obs = np.asarray(obs, dtype=np.float32)
    idx = np.asarray(neighbor_idx).astype(np.int64)

    enc_w65, Wq2, bq2, W2, b2 = _fold_weights(
        np.asarray(enc_w, np.float32), np.asarray(enc_b, np.float32),
        np.asarray(msg_w, np.float32), np.asarray(msg_b, np.float32),
        np.asarray(key_w, np.float32), np.asarray(key_b, np.float32),
        np.asarray(in_proj_w, np.float32), np.asarray(in_proj_b, np.float32),
        np.asarray(out_w, np.float32), np.asarray(out_b, np.float32),
        np.asarray(dec_w, np.float32), np.asarray(dec_b, np.float32))

    obs_b = obs.astype(bf)          # bf16 copy for the edge path
    ones_col = np.ones((1,), bf)

    in_maps = []
    for c in range(NCORES):
        base = c * SHARD
        obsT_shard = np.zeros((IN_DIM + 1, NS), bf)
        obsT_shard[:IN_DIM, :SHARD] = obs_b[base:base + SHARD].T
        obsT_shard[IN_DIM, :] = ones_col

        # per-edge obs gather, laid out so that column (j*EPC + u*128 + p)
        # holds obs[idx[node(j,g,p), k]] with u = g*16+k, node = (j*CSUP+g)*128+p
        sh_idx = np.zeros((NS, K), np.int64)
        sh_idx[:SHARD] = idx[base:base + SHARD]
        # cols as [j, g, k, p] -> value idx[(j*CSUP+g)*128+p, k]
        idx_r = sh_idx.reshape(NCHUNK, CSUP, P, K)          # [j, g, p, k]
        col_idx = idx_r.transpose(0, 1, 3, 2).reshape(-1)   # [j, g, k, p]
        og = obs_b[col_idx]                                 # [NS*K, 64] bf16
        obsgT = np.empty((IN_DIM + 1, NS * K), bf)
        obsgT[:IN_DIM] = og.T
        obsgT[IN_DIM] = ones_col

        wpack = np.zeros((P, 352), bf)
        wpack[:IN_DIM + 1, 0:H] = enc_w65.astype(bf)
        wpack[:, H:2 * H] = Wq2.astype(bf)
        wpack[:, 256:256 + A] = np.asarray(dec_w, np.float32).astype(bf)
        wpack[:, 288:288 + A] = W2.astype(bf)
        wpack[0, 320:320 + A] = b2.astype(bf)
        in_maps.append({
            "obsgT": obsgT, "obsT_shard": obsT_shard,
            "wpack": wpack, "bq2col": bq2[:, None].astype(np.float32),
        })
    return in_maps


def kernel(obs, neighbor_idx, enc_w, enc_b, msg_w, msg_b, key_w, key_b,
           in_proj_w, in_proj_b, out_w, out_b, dec_w, dec_b):
    from concourse import bass_utils

    in_maps = _prep_in_maps(
        obs, neighbor_idx, enc_w, enc_b, msg_w, msg_b, key_w, key_b,
        in_proj_w, in_proj_b, out_w, out_b, dec_w, dec_b)

    if "nc" not in _PROG_CACHE:
        _PROG_CACHE["nc"] = _build_program()
    nc = _PROG_CACHE["nc"]

    trace = bool(globals().get("_TRACE_RUN", False))
    res = bass_utils.run_bass_kernel_spmd(nc, in_maps, list(range(NCORES)),
                                          trace=trace)
    if trace:
        _PROG_CACHE["last_result"] = res

    out = np.empty((N, A), np.float32)
    for c in range(NCORES):
        o = np.asarray(res.results[c]["outp"], dtype=np.float32)
        o = o.reshape(P, NSUP, A).transpose(1, 0, 2)
        out[c * SHARD:(c + 1) * SHARD] = o.reshape(NS, A)[:SHARD]
    return out


# revision 28
# speedup vs baseline: 1.0668x; 1.0668x over previous
"""AgentNet GNN message-passing kernel for 8 Trainium2 NeuronCores.

Algebraic collapse (validated to 4e-7 rel err vs reference in fp32):
  hidden = relu(obs @ enc_w + enc_b)                      [N,128]
  q'     = hidden @ Wq2 + bq2      (q/k projections folded; key bias
                                    cancels in softmax)   [N,128]
  s[n,k] = q'[n] . hidden[idx[n,k]]
  attn   = softmax_k(s)            (scores are O(0.05) -> no max-subtract)
  y      = sum_k attn[n,k] * (hidden[idx[n,k]] @ W2)      [N,32]
  logits = hidden @ dec_w + y + b2
  out    = softmax_a(logits)

The neighbor gather commutes with the row-wise encoder, so the host gathers
obs rows per edge (cheap numpy take) and the device runs a dense per-edge
encoder matmul -- no data-dependent DMA on device.

v2 layout: the edge encoder emits TRANSPOSED hiddens gT[c,(k,p)] per
supertile (PE cost identical -- swap lhsT/rhs).  That moves both
reductions onto the idle PE:
  - scores: prod2[c,(k,p)] = gT * qT (DVE TT @2x), then 16 accumulating
    one-hot matmuls reduce over c -> sT[k,p] in PSUM; one PE transpose
    brings s back to node orientation for the softmax.
  - value: zg[p,(k,a)] = gT_k^T @ W2 on PE (a=32 wide, 4x smaller than
    hidden), then one DVE TT applies attn and a short k-tree finishes.
Engine knobs below split relu / tree / small ops across Act/DVE/Pool.

Sharding: nodes split into 8 contiguous shards (6250/core, padded to 6656).
Weights replicated; no collectives.
"""
import numpy as np

N, K = 50000, 16
IN_DIM, H, C, A = 64, 128, 128, 32
NCORES = 8
SHARD = N // NCORES              # 6250
P = 128
NSUP = 49                        # supertiles of 128 nodes per core (6272)
NS = NSUP * P                    # 6272 padded shard nodes (>= 6250)
CSUP = 4                         # supertiles per full chunk
CHUNKS = [(q * CSUP, CSUP) for q in range(NSUP // CSUP)] + [(48, 1)]
NCHUNK = len(CHUNKS)             # 12 full + 1 single-supertile tail
EPC = CSUP * P * K               # 8192 edges per full chunk

# ---- engine assignment (tuned against the TimelineSim cost model) ----
RELU_ENG = ("act", "act", "act", "act", "act", "act", "act", "dve",
            "act", "act", "act", "dve", "act", "act", "act", "act")
STAGEB_HID_ENG = "act"           # relu of hidT block
TREE_ENG = ("pool", "pool", "pool", "pool")   # k-tree levels 8,4,2,1
PR_ENG = "pool"                  # final prob scale
YS_ENG = "pool"                  # 1/Z value scale

_PROG_CACHE = {}


def _build_program():
    import concourse.bacc as bacc
    import concourse.mybir as mybir
    import concourse.tile as tile
    from concourse.masks import make_identity

    f32 = mybir.dt.float32
    bf16 = mybir.dt.bfloat16
    AX = mybir.AxisListType
    OP = mybir.AluOpType
    AF = mybir.ActivationFunctionType

    nc = bacc.Bacc("TRN2", target_bir_lowering=False, debug=False,
                   num_devices=1)

    # per-edge pre-gathered obs, transposed, with a trailing ones-row for the
    # encoder bias fold; column order: chunk j | supertile g | k | partition p
    obsgT = nc.dram_tensor("obsgT", [IN_DIM + 1, NS * K], bf16, kind="ExternalInput")
    obsT_shard = nc.dram_tensor("obsT_shard", [IN_DIM + 1, NS], bf16, kind="ExternalInput")
    # all bf16 weights packed into one tensor (single DMA at startup):
    # cols 0:128 enc_wb (rows 0:65) | 128:256 wq2 | 256:288 dec_w
    # | 288:320 w2 | 320:352 b2row (row 0)
    wpack = nc.dram_tensor("wpack", [P, 352], bf16, kind="ExternalInput")
    bq2col = nc.dram_tensor("bq2col", [C, 1], f32, kind="ExternalInput")
    outp = nc.dram_tensor("outp", [P, NSUP * A], bf16, kind="ExternalOutput")

    with tile.TileContext(nc) as tc:
        with tc.tile_pool(name="cst", bufs=1) as cst, \
             tc.tile_pool(name="obs", bufs=3) as obsp, \
             tc.tile_pool(name="big", bufs=1) as big, \
             tc.tile_pool(name="gts", bufs=4) as gts, \
             tc.tile_pool(name="pw", bufs=4) as pwp, \
             tc.tile_pool(name="sml", bufs=6) as sml, \
             tc.tile_pool(name="psG", bufs=2, space="PSUM") as psG, \
             tc.tile_pool(name="psS", bufs=2, space="PSUM") as psS, \
             tc.tile_pool(name="psZ", bufs=1, space="PSUM") as psZ, \
             tc.tile_pool(name="psL", bufs=1, space="PSUM") as psL:

            # ---- constants (one packed DMA for all bf16 weights) ----
            wpk = cst.tile([P, 352], bf16)
            nc.sync.dma_start(out=wpk[:], in_=wpack[:, :])
            encwb_sb = wpk[:IN_DIM + 1, 0:H]
            wq2_sb = wpk[:, H:2 * H]
            decw_sb = wpk[:, 256:256 + A]
            w2_sb = wpk[:, 288:288 + A]
            b2_sb = wpk[:1, 320:320 + A]
            bq2_sb = cst.tile([C, 1], f32)
            nc.sync.dma_start(out=bq2_sb[:], in_=bq2col[:, :])
            ones1 = cst.tile([1, P], bf16)
            nc.vector.memset(ones1[:], 1.0)
            onesPK = cst.tile([P, K], bf16)
            nc.vector.memset(onesPK[:], 1.0)
            ident = cst.tile([P, P], bf16)
            make_identity(nc, ident[:])
            identf = cst.tile([K, K], f32)
            make_identity(nc, identf[:])
            # onehot[:, k*16+j] = 1 iff j == k: lhsT for the per-k score
            # reduction (sum over c lands in psum partition k only)
            onehot = cst.tile([P, K * K], bf16)
            nc.vector.memset(onehot[:], 0.0)
            for k in range(K):
                nc.vector.memset(onehot[:, k * K + k:k * K + k + 1], 1.0)

            # persistent: transposed hidden + transposed q' for the shard
            hidT = big.tile([H, NS], bf16)
            qT = big.tile([C, NS], bf16)
            # decode-logit psum: ping-pong halves across chunks; the tail
            # 128 cols hold 8 rotating esc[p,16] slots (keeps the score-psum
            # tile's lifetime short -- it frees right after the exp)
            plbig = psL.tile([P, 2 * CSUP * A], f32, tag="pl",
                             space="PSUM")

            def open_chunk(ci, sup0, csup):
                nb = sup0 * P
                w = csup * P
                # stage B block: hidT, qT for nodes nb..nb+w
                obsb = obsp.tile([IN_DIM + 1, 512], bf16, tag="ob")
                nc.sync.dma_start(out=obsb[:, :w],
                                  in_=obsT_shard[:, nb:nb + w])
                phb = psS.tile([H, 512], f32, tag="sc", space="PSUM")
                ph = phb[:, :w]
                nc.tensor.matmul(out=ph, lhsT=encwb_sb, rhs=obsb[:, :w],
                                 start=True, stop=True)
                if STAGEB_HID_ENG == "act":
                    nc.scalar.activation(out=hidT[:, nb:nb + w], in_=ph,
                                         func=AF.Relu)
                else:
                    nc.vector.tensor_scalar_max(out=hidT[:, nb:nb + w],
                                                in0=ph, scalar1=0.0)
                pqb = psS.tile([C, 512], f32, tag="sc", space="PSUM")
                pq = pqb[:, :w]
                nc.tensor.matmul(out=pq, lhsT=wq2_sb,
                                 rhs=hidT[:, nb:nb + w],
                                 start=True, stop=True)
                # copy + query bias in one Act pass (bias is per-partition)
                nc.scalar.activation(out=qT[:, nb:nb + w], in_=pq,
                                     func=AF.Identity, bias=bq2_sb[:, :])

                og = obsp.tile([IN_DIM + 1, EPC], bf16, tag="og")
                for q4 in range(csup):
                    nc.sync.dma_start(
                        out=og[:, q4 * 2048:(q4 + 1) * 2048],
                        in_=obsgT[:, (sup0 + q4) * 2048:
                                  (sup0 + q4 + 1) * 2048])
                ychunk = sml.tile([P, CSUP * A], bf16, tag="y", name=f"y{ci}")
                zC = sml.tile([P, CSUP], f32, tag="zC", name=f"zC{ci}")
                rzC = sml.tile([P, CSUP], f32, tag="rzC", name=f"rzC{ci}")
                return {
                    "og": og, "sup0": sup0, "csup": csup,
                    "pl": plbig[:, (ci % 2) * CSUP * A:
                                (ci % 2) * CSUP * A + csup * A],
                    "y": ychunk,
                    "zC": zC,
                    "rzC": rzC,
                }

            def emit_edge(s, ctx):
                # edge encoder, transposed output: gT[c, (k,p)]
                gi = s - ctx["sup0"]
                og = ctx["og"]
                gt = gts.tile([C, K * P], bf16, tag="g")
                for h in range(2):
                    gps = psG.tile([C, 1024], f32, tag="gps", space="PSUM")
                    for q in range(2):
                        nc.tensor.matmul(
                            out=gps[:, q * 512:(q + 1) * 512],
                            lhsT=encwb_sb,
                            rhs=og[:, gi * 2048 + h * 1024 + q * 512:
                                   gi * 2048 + h * 1024 + (q + 1) * 512],
                            start=True, stop=True)
                    dst = gt[:, h * 1024:(h + 1) * 1024]
                    if RELU_ENG[(s * 2 + h) % len(RELU_ENG)] == "act":
                        nc.scalar.activation(out=dst, in_=gps[:], func=AF.Relu)
                    else:
                        nc.vector.tensor_scalar_max(out=dst, in0=gps[:],
                                                    scalar1=0.0)
                return gt

            def emit_value(s, ctx, gt):
                gi = s - ctx["sup0"]
                nb = ctx["sup0"] * P
                zC, rzC, ychunk, pl = ctx["zC"], ctx["rzC"], ctx["y"], ctx["pl"]
                # score products: prod2[c, k, p] = gT * qT (bcast over k),
                # split in k-halves so the PE reduce starts early
                prod2 = pwp.tile([C, K * P], bf16, tag="pw")
                p3 = prod2[:].rearrange("c (k p) -> c k p", p=P)
                g3 = gt[:].rearrange("c (k p) -> c k p", p=P)
                qb = qT[:, None, nb + gi * P:nb + (gi + 1) * P].to_broadcast(
                    [C, K, P])
                nc.vector.tensor_tensor(
                    out=p3[:, :8, :], in0=g3[:, :8, :],
                    in1=qb[:, :8, :], op=OP.mult)
                nc.vector.tensor_tensor(
                    out=p3[:, 8:, :], in0=g3[:, 8:, :],
                    in1=qb[:, 8:, :], op=OP.mult)

                # score reduce over c on PE: sT[k, p] via one-hot lhsT
                scps = psS.tile([P, 512], f32, tag="sc", space="PSUM")
                sT = scps[:K, :P]
                for k in range(K):
                    nc.tensor.matmul(
                        out=sT, lhsT=onehot[:, k * K:(k + 1) * K],
                        rhs=prod2[:, k * P:(k + 1) * P],
                        start=(k == 0), stop=(k == K - 1))
                # exp in k-orientation (fuses the psum drain with exp),
                # then transpose the exp'd scores to node orientation
                escT = sml.tile([K, P], f32, tag="sTsb")
                nc.scalar.activation(out=escT[:], in_=sT, func=AF.Exp)
                esc_ps = scps[:, P:P + K]
                nc.tensor.transpose(out=esc_ps, in_=escT[:],
                                    identity=identf[:K, :K])
                # drain esc PSUM->SBUF (HW allows only one PSUM operand per
                # TensorTensor and wp needs zg from PSUM), then the softmax
                # denominator off the critical path
                esc = sml.tile([P, K], bf16, tag="esc")
                nc.vector.tensor_scalar_mul(out=esc[:], in0=esc_ps,
                                            scalar1=1.0)
                nc.vector.reduce_sum(
                    out=zC[:, gi:gi + 1], in_=esc[:, None, :], axis=AX.X)
                nc.vector.reciprocal(out=rzC[:, gi:gi + 1],
                                     in_=zC[:, gi:gi + 1])

                # value path: zg[p, k, a] = gT_k^T @ W2 on PE
                zg = psZ.tile([P, K * A], f32, tag="zg", space="PSUM")
                for k in range(K):
                    nc.tensor.matmul(
                        out=zg[:, k * A:(k + 1) * A],
                        lhsT=gt[:, k * P:(k + 1) * P],
                        rhs=w2_sb, start=True, stop=True)
                wp = pwp.tile([P, K * A], bf16, tag="wp")
                w3 = wp[:].rearrange("p (k a) -> p k a", a=A)
                nc.vector.tensor_tensor(
                    out=w3[:, :, :],
                    in0=zg[:].rearrange("p (k a) -> p k a", a=A),
                    in1=esc[:, :, None].to_broadcast([P, K, A]),
                    op=OP.mult)
                # k-tree: 16 -> 8 -> 4 -> 2 -> 1, then 1/Z scale
                for li, w in enumerate((8, 4, 2)):
                    eng = nc.gpsimd if TREE_ENG[li] == "pool" else nc.vector
                    eng.tensor_tensor(
                        out=w3[:, :w, :], in0=w3[:, :w, :],
                        in1=w3[:, w:2 * w, :], op=OP.add)
                eng = nc.gpsimd if TREE_ENG[3] == "pool" else nc.vector
                eng.tensor_tensor(
                    out=w3[:, 0:1, :],
                    in0=w3[:, 0:1, :], in1=w3[:, 1:2, :], op=OP.add)
                yeng = nc.gpsimd if YS_ENG == "pool" else nc.vector
                yeng.tensor_scalar_mul(
                    out=ychunk[:, gi * A:(gi + 1) * A], in0=wp[:, :A],
                    scalar1=rzC[:, gi:gi + 1])

                # decode logits: hidden @ dec_w + b2 (y added below)
                sl = pl[:, gi * A:(gi + 1) * A]
                nc.tensor.matmul(out=sl, lhsT=hidT[:, s * P:(s + 1) * P],
                                 rhs=decw_sb, start=True, stop=False)
                nc.tensor.matmul(out=sl, lhsT=ones1[:], rhs=b2_sb,
                                 start=False, stop=True)

            def emit_tail(ctx):
                # decode softmax for the chunk
                sup0, csup = ctx["sup0"], ctx["csup"]
                wa = csup * A
                eli = sml.tile([P, CSUP * A], f32, tag="eli")
                nc.vector.tensor_tensor(out=eli[:, :wa], in0=ctx["pl"],
                                        in1=ctx["y"][:, :wa], op=OP.add)
                el = sml.tile([P, CSUP * A], bf16, tag="el")
                nc.scalar.activation(out=el[:, :wa], in_=eli[:, :wa],
                                     func=AF.Exp)
                zl = sml.tile([P, CSUP], f32, tag="zl")
                nc.vector.reduce_sum(
                    out=zl[:, :csup],
                    in_=el[:, :wa].rearrange("p (s a) -> p s a", a=A),
                    axis=AX.X)
                rzl = sml.tile([P, CSUP], f32, tag="rzl")
                nc.vector.reciprocal(out=rzl[:, :csup], in_=zl[:, :csup])
                pr = sml.tile([P, CSUP * A], bf16, tag="pr")
                eng = nc.gpsimd if PR_ENG == "pool" else nc.vector
                eng.tensor_tensor(
                    out=pr[:, :wa].rearrange("p (s a) -> p s a", a=A),
                    in0=el[:, :wa].rearrange("p (s a) -> p s a", a=A),
                    in1=rzl[:, :csup, None].to_broadcast([P, csup, A]),
                    op=OP.mult)
                nc.sync.dma_start(
                    out=outp[:, sup0 * A:(sup0 + csup) * A], in_=pr[:, :wa])

            # software-pipelined: edge(s) runs one supertile ahead of the
            # attention/value work so the PE never head-of-line blocks on the
            # single zg psum bank
            prev = None
            for ci, (sup0, csup) in enumerate(CHUNKS):
                ctx = open_chunk(ci, sup0, csup)
                for gi in range(csup):
                    s = sup0 + gi
                    gt = emit_edge(s, ctx)
                    if prev is not None:
                        emit_value(*prev)
                        if prev[0] == prev[1]["sup0"] + prev[1]["csup"] - 1:
                            emit_tail(prev[1])
                    prev = (s, ctx, gt)
            emit_value(*prev)
            emit_tail(prev[1])

    nc.compile()
    return nc


def _fold_weights(enc_w, enc_b, msg_w, msg_b, key_w, key_b,
                  in_proj_w, in_proj_b, out_w, out_b, dec_w, dec_b):
    wq, wk, wv = in_proj_w[:C], in_proj_w[C:2 * C], in_proj_w[2 * C:]
    bq, bv = in_proj_b[:C], in_proj_b[2 * C:]
    Wq_eff = msg_w @ wq.T
    bq_eff = msg_b @ wq.T + bq
    Wk_eff = key_w @ wk.T
    Wv_eff = msg_w @ wv.T
    bv_eff = msg_b @ wv.T + bv
    s = np.float32(1.0 / np.sqrt(np.float32(C)))
    Wq2 = (Wq_eff @ Wk_eff.T) * s
    bq2 = (bq_eff @ Wk_eff.T) * s
    W2 = Wv_eff @ out_w @ dec_w
    b2 = bv_eff @ out_w @ dec_w + out_b @ dec_w + dec_b
    enc_w65 = np.concatenate([enc_w, enc_b[None, :]], axis=0)
    return enc_w65.astype(np.float32), Wq2.astype(np.float32), \
        bq2.astype(np.float32), W2.astype(np.float32), b2.astype(np.float32)


def _prep_in_maps(obs, neighbor_idx, enc_w, enc_b, msg_w, msg_b, key_w,
                  key_b, in_proj_w, in_proj_b, out_w, out_b, dec_w, dec_b):
    import ml_dtypes

    bf = ml_dtypes.bfloat16
    obs = np.asarray(obs, dtype=np.float32)
    idx = np.asarray(neighbor_idx).astype(np.int64)

    enc_w65, Wq2, bq2, W2, b2 = _fold_weights(
        np.asarray(enc_w, np.float32), np.asarray(enc_b, np.float32),
        np.asarray(msg_w, np.float32), np.asarray(msg_b, np.float32),
        np.asarray(key_w, np.float32), np.asarray(key_b, np.float32),
        np.asarray(in_proj_w, np.float32), np.asarray(in_proj_b, np.float32),
        np.asarray(out_w, np.float32), np.asarray(out_b, np.float32),
        np.asarray(dec_w, np.float32), np.asarray(dec_b, np.float32))

    obs_b = obs.astype(bf)          # bf16 copy for the edge path
    ones_col = np.ones((1,), bf)

    in_maps = []
    for c in range(NCORES):
        base = c * SHARD
        obsT_shard = np.zeros((IN_DIM + 1, NS), bf)
        obsT_shard[:IN_DIM, :SHARD] = obs_b[base:base + SHARD].T
        obsT_shard[IN_DIM, :] = ones_col

        # per-edge obs gather: column (s*2048 + k*128 + p) holds
        # obs[idx[s*128+p, k]] (supertile-major, k, then partition)
        sh_idx = np.zeros((NS, K), np.int64)
        sh_idx[:SHARD] = idx[base:base + SHARD]
        idx_r = sh_idx.reshape(NSUP, P, K)                  # [s, p, k]
        col_idx = idx_r.transpose(0, 2, 1).reshape(-1)      # [s, k, p]
        og = obs_b[col_idx]                                 # [NS*K, 64] bf16
        obsgT = np.empty((IN_DIM + 1, NS * K), bf)
        obsgT[:IN_DIM] = og.T
        obsgT[IN_DIM] = ones_col

        wpack = np.zeros((P, 352), bf)
        wpack[:IN_DIM + 1, 0:H] = enc_w65.astype(bf)
        wpack[:, H:2 * H] = Wq2.astype(bf)
        wpack[:, 256:256 + A] = np.asarray(dec_w, np.float32).astype(bf)
        wpack[:, 288:288 + A] = W2.astype(bf)
        wpack[0, 320:320 + A] = b2.astype(bf)
        in_maps.append({
            "obsgT": obsgT, "obsT_shard": obsT_shard,
            "wpack": wpack, "bq2col": bq2[:, None].astype(np.float32),
        })
    return in_maps


def kernel(obs, neighbor_idx, enc_w, enc_b, msg_w, msg_b, key_w, key_b,
           in_proj_w, in_proj_b, out_w, out_b, dec_w, dec_b):
    from concourse import bass_utils

    in_maps = _prep_in_maps(
        obs, neighbor_idx, enc_w, enc_b, msg_w, msg_b, key_w, key_b,
        in_proj_w, in_proj_b, out_w, out_b, dec_w, dec_b)

    if "nc" not in _PROG_CACHE:
        _PROG_CACHE["nc"] = _build_program()
    nc = _PROG_CACHE["nc"]

    trace = bool(globals().get("_TRACE_RUN", False))
    res = bass_utils.run_bass_kernel_spmd(nc, in_maps, list(range(NCORES)),
                                          trace=trace)
    if trace:
        _PROG_CACHE["last_result"] = res

    out = np.empty((N, A), np.float32)
    for c in range(NCORES):
        o = np.asarray(res.results[c]["outp"], dtype=np.float32)
        o = o.reshape(P, NSUP, A).transpose(1, 0, 2)
        out[c * SHARD:(c + 1) * SHARD] = o.reshape(NS, A)[:SHARD]
    return out


# revision 31
# speedup vs baseline: 1.0676x; 1.0008x over previous
"""AgentNet GNN message-passing kernel for 8 Trainium2 NeuronCores.

Algebraic collapse (validated to 4e-7 rel err vs reference in fp32):
  hidden = relu(obs @ enc_w + enc_b)                      [N,128]
  q'     = hidden @ Wq2 + bq2      (q/k projections folded; key bias
                                    cancels in softmax)   [N,128]
  s[n,k] = q'[n] . hidden[idx[n,k]]
  attn   = softmax_k(s)            (scores are O(0.05) -> no max-subtract)
  y      = sum_k attn[n,k] * (hidden[idx[n,k]] @ W2)      [N,32]
  logits = hidden @ dec_w + y + b2
  out    = softmax_a(logits)

The neighbor gather commutes with the row-wise encoder, so the host gathers
obs rows per edge (cheap numpy take) and the device runs a dense per-edge
encoder matmul -- no data-dependent DMA on device.

v2 layout: the edge encoder emits TRANSPOSED hiddens gT[c,(k,p)] per
supertile (PE cost identical -- swap lhsT/rhs).  That moves both
reductions onto the idle PE:
  - scores: prod2[c,(k,p)] = gT * qT (DVE TT @2x), then 16 accumulating
    one-hot matmuls reduce over c -> sT[k,p] in PSUM; one PE transpose
    brings s back to node orientation for the softmax.
  - value: zg[p,(k,a)] = gT_k^T @ W2 on PE (a=32 wide, 4x smaller than
    hidden), then one DVE TT applies attn and a short k-tree finishes.
Engine knobs below split relu / tree / small ops across Act/DVE/Pool.

Sharding: nodes split into 8 contiguous shards (6250/core, padded to 6656).
Weights replicated; no collectives.
"""
import numpy as np

N, K = 50000, 16
IN_DIM, H, C, A = 64, 128, 128, 32
NCORES = 8
SHARD = N // NCORES              # 6250
P = 128
NSUP = 49                        # supertiles of 128 nodes per core (6272)
NS = NSUP * P                    # 6272 padded shard nodes (>= 6250)
CSUP = 4                         # supertiles per full chunk
CHUNKS = [(q * CSUP, CSUP) for q in range(NSUP // CSUP)] + [(48, 1)]
NCHUNK = len(CHUNKS)             # 12 full + 1 single-supertile tail
EPC = CSUP * P * K               # 8192 edges per full chunk

# ---- engine assignment (tuned against the TimelineSim cost model) ----
RELU_ENG = ("act", "act", "act", "act", "act", "act", "act", "dve",
            "act", "act", "act", "dve", "act", "act", "act", "act")
STAGEB_HID_ENG = "act"           # relu of hidT block
TREE_ENG = ("pool", "pool", "pool", "pool")   # k-tree levels 8,4,2,1
PR_ENG = "pool"                  # final prob scale
YS_ENG = "pool"                  # 1/Z value scale

_PROG_CACHE = {}


def _build_program():
    import concourse.bacc as bacc
    import concourse.mybir as mybir
    import concourse.tile as tile
    from concourse.masks import make_identity

    f32 = mybir.dt.float32
    bf16 = mybir.dt.bfloat16
    AX = mybir.AxisListType
    OP = mybir.AluOpType
    AF = mybir.ActivationFunctionType

    nc = bacc.Bacc("TRN2", target_bir_lowering=False, debug=False,
                   num_devices=1)

    # per-edge pre-gathered obs, transposed, with a trailing ones-row for the
    # encoder bias fold; column order: chunk j | supertile g | k | partition p
    obsgT = nc.dram_tensor("obsgT", [IN_DIM + 1, NS * K], bf16, kind="ExternalInput")
    obsT_shard = nc.dram_tensor("obsT_shard", [IN_DIM + 1, NS], bf16, kind="ExternalInput")
    # all bf16 weights packed into one tensor (single DMA at startup):
    # cols 0:128 enc_wb (rows 0:65) | 128:256 wq2 | 256:288 dec_w
    # | 288:320 w2 | 320:352 b2row (row 0)
    # cols 352:864 hold obsT block 0 so chunk 0's stage-B needs no extra DMA
    wpack = nc.dram_tensor("wpack", [P, 864], bf16, kind="ExternalInput")
    bq2col = nc.dram_tensor("bq2col", [C, 1], f32, kind="ExternalInput")
    outp = nc.dram_tensor("outp", [P, NSUP * A], bf16, kind="ExternalOutput")

    with tile.TileContext(nc) as tc:
        with tc.tile_pool(name="cst", bufs=1) as cst, \
             tc.tile_pool(name="obs", bufs=3) as obsp, \
             tc.tile_pool(name="big", bufs=1) as big, \
             tc.tile_pool(name="gts", bufs=4) as gts, \
             tc.tile_pool(name="pw", bufs=4) as pwp, \
             tc.tile_pool(name="sml", bufs=6) as sml, \
             tc.tile_pool(name="psG", bufs=2, space="PSUM") as psG, \
             tc.tile_pool(name="psS", bufs=2, space="PSUM") as psS, \
             tc.tile_pool(name="psZ", bufs=1, space="PSUM") as psZ, \
             tc.tile_pool(name="psL", bufs=1, space="PSUM") as psL:

            # ---- constants (one packed DMA for all bf16 weights) ----
            wpk = cst.tile([P, 864], bf16)
            nc.sync.dma_start(out=wpk[:], in_=wpack[:, :])
            encwb_sb = wpk[:IN_DIM + 1, 0:H]
            wq2_sb = wpk[:, H:2 * H]
            decw_sb = wpk[:, 256:256 + A]
            w2_sb = wpk[:, 288:288 + A]
            b2_sb = wpk[:1, 320:320 + A]
            bq2_sb = cst.tile([C, 1], f32)
            nc.sync.dma_start(out=bq2_sb[:], in_=bq2col[:, :])
            ones1 = cst.tile([1, P], bf16)
            nc.vector.memset(ones1[:], 1.0)
            onesPK = cst.tile([P, K], bf16)
            nc.vector.memset(onesPK[:], 1.0)
            ident = cst.tile([P, P], bf16)
            make_identity(nc, ident[:])
            identf = cst.tile([K, K], f32)
            make_identity(nc, identf[:])
            # onehot[:, k*16+j] = 1 iff j == k: lhsT for the per-k score
            # reduction (sum over c lands in psum partition k only)
            onehot = cst.tile([P, K * K], bf16)
            nc.vector.memset(onehot[:], 0.0)
            for k in range(K):
                nc.vector.memset(onehot[:, k * K + k:k * K + k + 1], 1.0)

            # persistent: transposed hidden + transposed q' for the shard
            hidT = big.tile([H, NS], bf16)
            qT = big.tile([C, NS], bf16)
            # decode-logit psum: ping-pong halves across chunks; the tail
            # 128 cols hold 8 rotating esc[p,16] slots (keeps the score-psum
            # tile's lifetime short -- it frees right after the exp)
            plbig = psL.tile([P, 2 * CSUP * A], f32, tag="pl",
                             space="PSUM")

            def open_chunk(ci, sup0, csup):
                nb = sup0 * P
                w = csup * P
                # stage B block: hidT, qT for nodes nb..nb+w
                if ci == 0:
                    obsb = wpk[:IN_DIM + 1, 352:864]
                else:
                    obsbt = obsp.tile([IN_DIM + 1, 512], bf16, tag="ob")
                    nc.sync.dma_start(out=obsbt[:, :w],
                                      in_=obsT_shard[:, nb:nb + w])
                    obsb = obsbt[:, :512]
                phb = psS.tile([H, 512], f32, tag="sc", space="PSUM")
                ph = phb[:, :w]
                nc.tensor.matmul(out=ph, lhsT=encwb_sb, rhs=obsb[:, :w],
                                 start=True, stop=True)
                if STAGEB_HID_ENG == "act":
                    nc.scalar.activation(out=hidT[:, nb:nb + w], in_=ph,
                                         func=AF.Relu)
                else:
                    nc.vector.tensor_scalar_max(out=hidT[:, nb:nb + w],
                                                in0=ph, scalar1=0.0)
                pqb = psS.tile([C, 512], f32, tag="sc", space="PSUM")
                pq = pqb[:, :w]
                nc.tensor.matmul(out=pq, lhsT=wq2_sb,
                                 rhs=hidT[:, nb:nb + w],
                                 start=True, stop=True)
                # copy + query bias in one Act pass (bias is per-partition)
                nc.scalar.activation(out=qT[:, nb:nb + w], in_=pq,
                                     func=AF.Identity, bias=bq2_sb[:, :])

                og = obsp.tile([IN_DIM + 1, EPC], bf16, tag="og")
                for q4 in range(csup):
                    nc.sync.dma_start(
                        out=og[:, q4 * 2048:(q4 + 1) * 2048],
                        in_=obsgT[:, (sup0 + q4) * 2048:
                                  (sup0 + q4 + 1) * 2048])
                ychunk = sml.tile([P, CSUP * A], bf16, tag="y", name=f"y{ci}")
                zC = sml.tile([P, CSUP], f32, tag="zC", name=f"zC{ci}")
                rzC = sml.tile([P, CSUP], f32, tag="rzC", name=f"rzC{ci}")
                return {
                    "og": og, "sup0": sup0, "csup": csup,
                    "pl": plbig[:, (ci % 2) * CSUP * A:
                                (ci % 2) * CSUP * A + csup * A],
                    "y": ychunk,
                    "zC": zC,
                    "rzC": rzC,
                }

            def emit_edge(s, ctx):
                # edge encoder, transposed output: gT[c, (k,p)]
                gi = s - ctx["sup0"]
                og = ctx["og"]
                gt = gts.tile([C, K * P], bf16, tag="g")
                for h in range(2):
                    gps = psG.tile([C, 1024], f32, tag="gps", space="PSUM")
                    for q in range(2):
                        nc.tensor.matmul(
                            out=gps[:, q * 512:(q + 1) * 512],
                            lhsT=encwb_sb,
                            rhs=og[:, gi * 2048 + h * 1024 + q * 512:
                                   gi * 2048 + h * 1024 + (q + 1) * 512],
                            start=True, stop=True)
                    dst = gt[:, h * 1024:(h + 1) * 1024]
                    if RELU_ENG[(s * 2 + h) % len(RELU_ENG)] == "act":
                        nc.scalar.activation(out=dst, in_=gps[:], func=AF.Relu)
                    else:
                        nc.vector.tensor_scalar_max(out=dst, in0=gps[:],
                                                    scalar1=0.0)
                return gt

            def emit_value(s, ctx, gt):
                gi = s - ctx["sup0"]
                nb = ctx["sup0"] * P
                zC, rzC, ychunk, pl = ctx["zC"], ctx["rzC"], ctx["y"], ctx["pl"]
                # score products: prod2[c, k, p] = gT * qT (bcast over k),
                # split in k-halves so the PE reduce starts early
                prod2 = pwp.tile([C, K * P], bf16, tag="pw")
                p3 = prod2[:].rearrange("c (k p) -> c k p", p=P)
                g3 = gt[:].rearrange("c (k p) -> c k p", p=P)
                qb = qT[:, None, nb + gi * P:nb + (gi + 1) * P].to_broadcast(
                    [C, K, P])
                nc.vector.tensor_tensor(
                    out=p3[:, :8, :], in0=g3[:, :8, :],
                    in1=qb[:, :8, :], op=OP.mult)
                nc.vector.tensor_tensor(
                    out=p3[:, 8:, :], in0=g3[:, 8:, :],
                    in1=qb[:, 8:, :], op=OP.mult)

                # score reduce over c on PE: sT[k, p] via one-hot lhsT
                scps = psS.tile([P, 512], f32, tag="sc", space="PSUM")
                sT = scps[:K, :P]
                for k in range(K):
                    nc.tensor.matmul(
                        out=sT, lhsT=onehot[:, k * K:(k + 1) * K],
                        rhs=prod2[:, k * P:(k + 1) * P],
                        start=(k == 0), stop=(k == K - 1))
                # exp in k-orientation (fuses the psum drain with exp),
                # then transpose the exp'd scores to node orientation
                escT = sml.tile([K, P], f32, tag="sTsb")
                nc.scalar.activation(out=escT[:], in_=sT, func=AF.Exp)
                esc_ps = scps[:, P:P + K]
                nc.tensor.transpose(out=esc_ps, in_=escT[:],
                                    identity=identf[:K, :K])
                # drain esc PSUM->SBUF (HW allows only one PSUM operand per
                # TensorTensor and wp needs zg from PSUM), then the softmax
                # denominator off the critical path
                esc = sml.tile([P, K], bf16, tag="esc")
                nc.vector.tensor_scalar_mul(out=esc[:], in0=esc_ps,
                                            scalar1=1.0)
                nc.vector.reduce_sum(
                    out=zC[:, gi:gi + 1], in_=esc[:, None, :], axis=AX.X)
                nc.vector.reciprocal(out=rzC[:, gi:gi + 1],
                                     in_=zC[:, gi:gi + 1])

                # value path: zg[p, k, a] = gT_k^T @ W2 on PE
                zg = psZ.tile([P, K * A], f32, tag="zg", space="PSUM")
                for k in range(K):
                    nc.tensor.matmul(
                        out=zg[:, k * A:(k + 1) * A],
                        lhsT=gt[:, k * P:(k + 1) * P],
                        rhs=w2_sb, start=True, stop=True)
                wp = pwp.tile([P, K * A], bf16, tag="wp")
                w3 = wp[:].rearrange("p (k a) -> p k a", a=A)
                nc.vector.tensor_tensor(
                    out=w3[:, :, :],
                    in0=zg[:].rearrange("p (k a) -> p k a", a=A),
                    in1=esc[:, :, None].to_broadcast([P, K, A]),
                    op=OP.mult)
                # k-tree: 16 -> 8 -> 4 -> 2 -> 1, then 1/Z scale
                # (last supertile stays on DVE: shorter drain chain)
                last = s == NSUP - 1
                for li, w in enumerate((8, 4, 2)):
                    pool_l = TREE_ENG[li] == "pool" and not last
                    eng = nc.gpsimd if pool_l else nc.vector
                    eng.tensor_tensor(
                        out=w3[:, :w, :], in0=w3[:, :w, :],
                        in1=w3[:, w:2 * w, :], op=OP.add)
                eng = nc.gpsimd if TREE_ENG[3] == "pool" and not last \
                    else nc.vector
                eng.tensor_tensor(
                    out=w3[:, 0:1, :],
                    in0=w3[:, 0:1, :], in1=w3[:, 1:2, :], op=OP.add)
                yeng = nc.gpsimd if YS_ENG == "pool" and not last \
                    else nc.vector
                yeng.tensor_scalar_mul(
                    out=ychunk[:, gi * A:(gi + 1) * A], in0=wp[:, :A],
                    scalar1=rzC[:, gi:gi + 1])

                # decode logits: hidden @ dec_w + b2 (y added below)
                sl = pl[:, gi * A:(gi + 1) * A]
                nc.tensor.matmul(out=sl, lhsT=hidT[:, s * P:(s + 1) * P],
                                 rhs=decw_sb, start=True, stop=False)
                nc.tensor.matmul(out=sl, lhsT=ones1[:], rhs=b2_sb,
                                 start=False, stop=True)

            def emit_tail(ctx):
                # decode softmax for the chunk
                sup0, csup = ctx["sup0"], ctx["csup"]
                wa = csup * A
                eli = sml.tile([P, CSUP * A], f32, tag="eli")
                nc.vector.tensor_tensor(out=eli[:, :wa], in0=ctx["pl"],
                                        in1=ctx["y"][:, :wa], op=OP.add)
                el = sml.tile([P, CSUP * A], bf16, tag="el")
                nc.scalar.activation(out=el[:, :wa], in_=eli[:, :wa],
                                     func=AF.Exp)
                zl = sml.tile([P, CSUP], f32, tag="zl")
                nc.vector.reduce_sum(
                    out=zl[:, :csup],
                    in_=el[:, :wa].rearrange("p (s a) -> p s a", a=A),
                    axis=AX.X)
                rzl = sml.tile([P, CSUP], f32, tag="rzl")
                nc.vector.reciprocal(out=rzl[:, :csup], in_=zl[:, :csup])
                pr = sml.tile([P, CSUP * A], bf16, tag="pr")
                eng = nc.gpsimd if PR_ENG == "pool" and csup > 1 \
                    else nc.vector
                eng.tensor_tensor(
                    out=pr[:, :wa].rearrange("p (s a) -> p s a", a=A),
                    in0=el[:, :wa].rearrange("p (s a) -> p s a", a=A),
                    in1=rzl[:, :csup, None].to_broadcast([P, csup, A]),
                    op=OP.mult)
                nc.sync.dma_start(
                    out=outp[:, sup0 * A:(sup0 + csup) * A], in_=pr[:, :wa])

            # software-pipelined: edge(s) runs one supertile ahead of the
            # attention/value work so the PE never head-of-line blocks on the
            # single zg psum bank
            prev = None
            for ci, (sup0, csup) in enumerate(CHUNKS):
                ctx = open_chunk(ci, sup0, csup)
                for gi in range(csup):
                    s = sup0 + gi
                    gt = emit_edge(s, ctx)
                    if prev is not None:
                        emit_value(*prev)
                        if prev[0] == prev[1]["sup0"] + prev[1]["csup"] - 1:
                            emit_tail(prev[1])
                    prev = (s, ctx, gt)
            emit_value(*prev)
            emit_tail(prev[1])

    nc.compile()
    return nc


def _fold_weights(enc_w, enc_b, msg_w, msg_b, key_w, key_b,
                  in_proj_w, in_proj_b, out_w, out_b, dec_w, dec_b):
    wq, wk, wv = in_proj_w[:C], in_proj_w[C:2 * C], in_proj_w[2 * C:]
    bq, bv = in_proj_b[:C], in_proj_b[2 * C:]
    Wq_eff = msg_w @ wq.T
    bq_eff = msg_b @ wq.T + bq
    Wk_eff = key_w @ wk.T
    Wv_eff = msg_w @ wv.T
    bv_eff = msg_b @ wv.T + bv
    s = np.float32(1.0 / np.sqrt(np.float32(C)))
    Wq2 = (Wq_eff @ Wk_eff.T) * s
    bq2 = (bq_eff @ Wk_eff.T) * s
    W2 = Wv_eff @ out_w @ dec_w
    b2 = bv_eff @ out_w @ dec_w + out_b @ dec_w + dec_b
    enc_w65 = np.concatenate([enc_w, enc_b[None, :]], axis=0)
    return enc_w65.astype(np.float32), Wq2.astype(np.float32), \
        bq2.astype(np.float32), W2.astype(np.float32), b2.astype(np.float32)


def _prep_in_maps(obs, neighbor_idx, enc_w, enc_b, msg_w, msg_b, key_w,
                  key_b, in_proj_w, in_proj_b, out_w, out_b, dec_w, dec_b):
    import ml_dtypes

    bf = ml_dtypes.bfloat16
    obs = np.asarray(obs, dtype=np.float32)
    idx = np.asarray(neighbor_idx).astype(np.int64)

    enc_w65, Wq2, bq2, W2, b2 = _fold_weights(
        np.asarray(enc_w, np.float32), np.asarray(enc_b, np.float32),
        np.asarray(msg_w, np.float32), np.asarray(msg_b, np.float32),
        np.asarray(key_w, np.float32), np.asarray(key_b, np.float32),
        np.asarray(in_proj_w, np.float32), np.asarray(in_proj_b, np.float32),
        np.asarray(out_w, np.float32), np.asarray(out_b, np.float32),
        np.asarray(dec_w, np.float32), np.asarray(dec_b, np.float32))

    obs_b = obs.astype(bf)          # bf16 copy for the edge path
    ones_col = np.ones((1,), bf)

    in_maps = []
    for c in range(NCORES):
        base = c * SHARD
        obsT_shard = np.zeros((IN_DIM + 1, NS), bf)
        obsT_shard[:IN_DIM, :SHARD] = obs_b[base:base + SHARD].T
        obsT_shard[IN_DIM, :] = ones_col

        # per-edge obs gather: column (s*2048 + k*128 + p) holds
        # obs[idx[s*128+p, k]] (supertile-major, k, then partition)
        sh_idx = np.zeros((NS, K), np.int64)
        sh_idx[:SHARD] = idx[base:base + SHARD]
        idx_r = sh_idx.reshape(NSUP, P, K)                  # [s, p, k]
        col_idx = idx_r.transpose(0, 2, 1).reshape(-1)      # [s, k, p]
        og = obs_b[col_idx]                                 # [NS*K, 64] bf16
        obsgT = np.empty((IN_DIM + 1, NS * K), bf)
        obsgT[:IN_DIM] = og.T
        obsgT[IN_DIM] = ones_col

        wpack = np.zeros((P, 864), bf)
        wpack[:IN_DIM + 1, 0:H] = enc_w65.astype(bf)
        wpack[:, H:2 * H] = Wq2.astype(bf)
        wpack[:, 256:256 + A] = np.asarray(dec_w, np.float32).astype(bf)
        wpack[:, 288:288 + A] = W2.astype(bf)
        wpack[0, 320:320 + A] = b2.astype(bf)
        wpack[:IN_DIM + 1, 352:864] = obsT_shard[:, :512]
        in_maps.append({
            "obsgT": obsgT, "obsT_shard": obsT_shard,
            "wpack": wpack, "bq2col": bq2[:, None].astype(np.float32),
        })
    return in_maps


def kernel(obs, neighbor_idx, enc_w, enc_b, msg_w, msg_b, key_w, key_b,
           in_proj_w, in_proj_b, out_w, out_b, dec_w, dec_b):
    from concourse import bass_utils

    in_maps = _prep_in_maps(
        obs, neighbor_idx, enc_w, enc_b, msg_w, msg_b, key_w, key_b,
        in_proj_w, in_proj_b, out_w, out_b, dec_w, dec_b)

    if "nc" not in _PROG_CACHE:
        _PROG_CACHE["nc"] = _build_program()
    nc = _PROG_CACHE["nc"]

    trace = bool(globals().get("_TRACE_RUN", False))
    res = bass_utils.run_bass_kernel_spmd(nc, in_maps, list(range(NCORES)),
                                          trace=trace)
    if trace:
        _PROG_CACHE["last_result"] = res

    out = np.empty((N, A), np.float32)
    for c in range(NCORES):
        o = np.asarray(res.results[c]["outp"], dtype=np.float32)
        o = o.reshape(P, NSUP, A).transpose(1, 0, 2)
        out[c * SHARD:(c + 1) * SHARD] = o.reshape(NS, A)[:SHARD]
    return out


# revision 41
# speedup vs baseline: 1.0743x; 1.0063x over previous
"""AgentNet GNN message-passing kernel for 8 Trainium2 NeuronCores.

Algebraic collapse (validated to 4e-7 rel err vs reference in fp32):
  hidden = relu(obs @ enc_w + enc_b)                      [N,128]
  q'     = hidden @ Wq2 + bq2      (q/k projections folded; key bias
                                    cancels in softmax)   [N,128]
  s[n,k] = q'[n] . hidden[idx[n,k]]
  attn   = softmax_k(s)            (scores are O(0.05) -> no max-subtract)
  y      = sum_k attn[n,k] * (hidden[idx[n,k]] @ W2)      [N,32]
  logits = hidden @ dec_w + y + b2
  out    = softmax_a(logits)

The neighbor gather commutes with the row-wise encoder, so the host gathers
obs rows per edge (cheap numpy take) and the device runs a dense per-edge
encoder matmul -- no data-dependent DMA on device.

v2 layout: the edge encoder emits TRANSPOSED hiddens gT[c,(k,p)] per
supertile (PE cost identical -- swap lhsT/rhs).  That moves both
reductions onto the idle PE:
  - scores: prod2[c,(k,p)] = gT * qT (DVE TT @2x), then 16 accumulating
    one-hot matmuls reduce over c -> sT[k,p] in PSUM; one PE transpose
    brings s back to node orientation for the softmax.
  - value: zg[p,(k,a)] = gT_k^T @ W2 on PE (a=32 wide, 4x smaller than
    hidden), then one DVE TT applies attn and a short k-tree finishes.
Engine knobs below split relu / tree / small ops across Act/DVE/Pool.

Sharding: nodes split into 8 contiguous shards (6250/core, padded to 6656).
Weights replicated; no collectives.
"""
import numpy as np

N, K = 50000, 16
IN_DIM, H, C, A = 64, 128, 128, 32
NCORES = 8
SHARD = N // NCORES              # 6250
P = 128
NSUP = 49                        # supertiles of 128 nodes per core (6272)
NS = NSUP * P                    # 6272 padded shard nodes (>= 6250)
CSUP = 4                         # supertiles per full chunk
CHUNKS = [(q * CSUP, CSUP) for q in range(NSUP // CSUP)] + [(48, 1)]
NCHUNK = len(CHUNKS)             # 12 full + 1 single-supertile tail
EPC = CSUP * P * K               # 8192 edges per full chunk

# ---- engine assignment (tuned against the TimelineSim cost model) ----
RELU_ENG = ("act", "act", "act", "act", "act", "act", "act", "dve",
            "act", "act", "act", "dve", "act", "act", "act", "act")
STAGEB_HID_ENG = "act"           # relu of hidT block
TREE_ENG = ("pool", "pool", "pool", "pool")   # k-tree levels 8,4,2,1
PR_ENG = "pool"                  # final prob scale
YS_ENG = "pool"                  # 1/Z value scale

_PROG_CACHE = {}


def _build_program():
    import concourse.bacc as bacc
    import concourse.mybir as mybir
    import concourse.tile as tile
    from concourse.masks import make_identity

    f32 = mybir.dt.float32
    bf16 = mybir.dt.bfloat16
    AX = mybir.AxisListType
    OP = mybir.AluOpType
    AF = mybir.ActivationFunctionType

    nc = bacc.Bacc("TRN2", target_bir_lowering=False, debug=False,
                   num_devices=1)

    # per-edge pre-gathered obs, transposed, with a trailing ones-row for the
    # encoder bias fold; column order: chunk j | supertile g | k | partition p
    obsgT = nc.dram_tensor("obsgT", [IN_DIM + 1, NS * K], bf16, kind="ExternalInput")
    obsT_shard = nc.dram_tensor("obsT_shard", [IN_DIM + 1, NS], bf16, kind="ExternalInput")
    # all bf16 weights packed into one tensor (single DMA at startup):
    # cols 0:128 enc_wb (rows 0:65) | 128:256 wq2 | 256:288 dec_w
    # | 288:320 w2 | 320:352 b2row (row 0)
    # cols 352:864 hold obsT block 0 so chunk 0's stage-B needs no extra
    # DMA; col 864 is the folded query bias (bf16 is plenty for a bias)
    wpack = nc.dram_tensor("wpack", [P, 865], bf16, kind="ExternalInput")
    outp = nc.dram_tensor("outp", [P, NSUP * A], bf16, kind="ExternalOutput")

    with tile.TileContext(nc) as tc:
        with tc.tile_pool(name="cst", bufs=1) as cst, \
             tc.tile_pool(name="obs", bufs=3) as obsp, \
             tc.tile_pool(name="big", bufs=1) as big, \
             tc.tile_pool(name="gts", bufs=4) as gts, \
             tc.tile_pool(name="pw", bufs=4) as pwp, \
             tc.tile_pool(name="sml", bufs=6) as sml, \
             tc.tile_pool(name="psG", bufs=2, space="PSUM") as psG, \
             tc.tile_pool(name="psS", bufs=2, space="PSUM") as psS, \
             tc.tile_pool(name="psZ", bufs=1, space="PSUM") as psZ, \
             tc.tile_pool(name="psL", bufs=1, space="PSUM") as psL:

            # ---- constants (one packed DMA for all bf16 weights) ----
            wpk = cst.tile([P, 865], bf16)
            nc.sync.dma_start(out=wpk[:], in_=wpack[:, :])
            encwb_sb = wpk[:IN_DIM + 1, 0:H]
            wq2_sb = wpk[:, H:2 * H]
            decw_sb = wpk[:, 256:256 + A]
            w2_sb = wpk[:, 288:288 + A]
            b2_sb = wpk[:1, 320:320 + A]
            bq2_sb = wpk[:, 864:865]

            ones1 = cst.tile([1, P], bf16)
            nc.vector.memset(ones1[:], 1.0)
            onesPK = cst.tile([P, K], bf16)
            nc.vector.memset(onesPK[:], 1.0)
            ident = cst.tile([P, P], bf16)
            make_identity(nc, ident[:])
            identf = cst.tile([K, K], f32)
            make_identity(nc, identf[:])
            # onehot[:, k*16+j] = 1 iff j == k: lhsT for the per-k score
            # reduction (sum over c lands in psum partition k only)
            onehot = cst.tile([P, K * K], bf16)
            nc.vector.memset(onehot[:], 0.0)
            for k in range(K):
                nc.vector.memset(onehot[:, k * K + k:k * K + k + 1], 1.0)

            # persistent: transposed hidden + transposed q' for the shard
            hidT = big.tile([H, NS], bf16)
            qT = big.tile([C, NS], bf16)
            # decode-logit psum: ping-pong halves across chunks; the tail
            # 128 cols hold 8 rotating esc[p,16] slots (keeps the score-psum
            # tile's lifetime short -- it frees right after the exp)
            plbig = psL.tile([P, 2 * CSUP * A], f32, tag="pl",
                             space="PSUM")

            def open_chunk(ci, sup0, csup):
                nb = sup0 * P
                w = csup * P
                # stage B block: hidT, qT for nodes nb..nb+w
                if ci == 0:
                    obsb = wpk[:IN_DIM + 1, 352:864]
                else:
                    obsbt = obsp.tile([IN_DIM + 1, 512], bf16, tag="ob")
                    nc.sync.dma_start(out=obsbt[:, :w],
                                      in_=obsT_shard[:, nb:nb + w])
                    obsb = obsbt[:, :512]
                if False:
                    pass
                else:
                    phb = psS.tile([H, 512], f32, tag="sc", space="PSUM")
                    ph = phb[:, :w]
                    nc.tensor.matmul(out=ph, lhsT=encwb_sb, rhs=obsb[:, :w],
                                     start=True, stop=True)
                    if STAGEB_HID_ENG == "act":
                        nc.scalar.activation(out=hidT[:, nb:nb + w], in_=ph,
                                             func=AF.Relu)
                    else:
                        nc.vector.tensor_scalar_max(out=hidT[:, nb:nb + w],
                                                    in0=ph, scalar1=0.0)
                    pqb = psS.tile([C, 512], f32, tag="sc", space="PSUM")
                    pq = pqb[:, :w]
                    nc.tensor.matmul(out=pq, lhsT=wq2_sb,
                                     rhs=hidT[:, nb:nb + w],
                                     start=True, stop=True)
                    # copy + query bias in one Act pass (bias per-partition)
                    nc.scalar.activation(out=qT[:, nb:nb + w], in_=pq,
                                         func=AF.Identity, bias=bq2_sb)

                og = obsp.tile([IN_DIM + 1, EPC], bf16, tag="og")
                for q4 in range(csup):
                    nc.sync.dma_start(
                        out=og[:, q4 * 2048:(q4 + 1) * 2048],
                        in_=obsgT[:, (sup0 + q4) * 2048:
                                  (sup0 + q4 + 1) * 2048])
                ychunk = sml.tile([P, CSUP * A], bf16, tag="y", name=f"y{ci}")
                zC = sml.tile([P, CSUP], f32, tag="zC", name=f"zC{ci}")
                rzC = sml.tile([P, CSUP], f32, tag="rzC", name=f"rzC{ci}")
                return {
                    "og": og, "sup0": sup0, "csup": csup,
                    "pl": plbig[:, (ci % 2) * CSUP * A:
                                (ci % 2) * CSUP * A + csup * A],
                    "y": ychunk,
                    "zC": zC,
                    "rzC": rzC,
                }

            def emit_edge(s, ctx):
                # edge encoder, transposed output: gT[c, (k,p)]
                gi = s - ctx["sup0"]
                og = ctx["og"]
                gt = gts.tile([C, K * P], bf16, tag="g")
                for h in range(2):
                    gps = psG.tile([C, 1024], f32, tag="gps", space="PSUM")
                    for q in range(2):
                        nc.tensor.matmul(
                            out=gps[:, q * 512:(q + 1) * 512],
                            lhsT=encwb_sb,
                            rhs=og[:, gi * 2048 + h * 1024 + q * 512:
                                   gi * 2048 + h * 1024 + (q + 1) * 512],
                            start=True, stop=True)
                    dst = gt[:, h * 1024:(h + 1) * 1024]
                    if RELU_ENG[(s * 2 + h) % len(RELU_ENG)] == "act":
                        nc.scalar.activation(out=dst, in_=gps[:], func=AF.Relu)
                    else:
                        nc.vector.tensor_scalar_max(out=dst, in0=gps[:],
                                                    scalar1=0.0)
                return gt

            def emit_value(s, ctx, gt):
                gi = s - ctx["sup0"]
                nb = ctx["sup0"] * P
                zC, rzC, ychunk, pl = ctx["zC"], ctx["rzC"], ctx["y"], ctx["pl"]
                # score products: prod2[c, k, p] = gT * qT (bcast over k),
                # split in k-halves so the PE reduce starts early
                prod2 = pwp.tile([C, K * P], bf16, tag="pw")
                p3 = prod2[:].rearrange("c (k p) -> c k p", p=P)
                g3 = gt[:].rearrange("c (k p) -> c k p", p=P)
                qb = qT[:, None, nb + gi * P:nb + (gi + 1) * P].to_broadcast(
                    [C, K, P])
                nc.vector.tensor_tensor(
                    out=p3[:, :8, :], in0=g3[:, :8, :],
                    in1=qb[:, :8, :], op=OP.mult)
                nc.vector.tensor_tensor(
                    out=p3[:, 8:, :], in0=g3[:, 8:, :],
                    in1=qb[:, 8:, :], op=OP.mult)

                # score reduce over c on PE: sT[k, p] via one-hot lhsT
                scps = psS.tile([P, 512], f32, tag="sc", space="PSUM")
                sT = scps[:K, :P]
                for k in range(K):
                    nc.tensor.matmul(
                        out=sT, lhsT=onehot[:, k * K:(k + 1) * K],
                        rhs=prod2[:, k * P:(k + 1) * P],
                        start=(k == 0), stop=(k == K - 1))
                # exp in k-orientation (fuses the psum drain with exp),
                # then transpose the exp'd scores to node orientation
                escT = sml.tile([K, P], f32, tag="sTsb")
                with tc.high_priority(offset=40):
                    nc.scalar.activation(out=escT[:], in_=sT, func=AF.Exp)
                    esc_ps = scps[:, P:P + K]
                    nc.tensor.transpose(out=esc_ps, in_=escT[:],
                                        identity=identf[:K, :K])
                # drain esc PSUM->SBUF (HW allows only one PSUM operand per
                # TensorTensor and wp needs zg from PSUM), then the softmax
                # denominator off the critical path
                esc = sml.tile([P, K], bf16, tag="esc")
                with tc.high_priority(offset=40):
                    nc.vector.tensor_scalar_mul(out=esc[:], in0=esc_ps,
                                                scalar1=1.0)
                nc.vector.reduce_sum(
                    out=zC[:, gi:gi + 1], in_=esc[:, None, :], axis=AX.X)
                nc.vector.reciprocal(out=rzC[:, gi:gi + 1],
                                     in_=zC[:, gi:gi + 1])

                # value path: zg[p, k, a] = gT_k^T @ W2 on PE
                zg = psZ.tile([P, K * A], f32, tag="zg", space="PSUM")
                for k in range(K):
                    nc.tensor.matmul(
                        out=zg[:, k * A:(k + 1) * A],
                        lhsT=gt[:, k * P:(k + 1) * P],
                        rhs=w2_sb, start=True, stop=True)
                wp = pwp.tile([P, K * A], bf16, tag="wp")
                w3 = wp[:].rearrange("p (k a) -> p k a", a=A)
                nc.vector.tensor_tensor(
                    out=w3[:, :, :],
                    in0=zg[:].rearrange("p (k a) -> p k a", a=A),
                    in1=esc[:, :, None].to_broadcast([P, K, A]),
                    op=OP.mult)
                # k-tree: 16 -> 8 -> 4 -> 2 -> 1, then 1/Z scale
                # (last supertile stays on DVE: shorter drain chain)
                last = s == NSUP - 1
                for li, w in enumerate((8, 4, 2)):
                    pool_l = TREE_ENG[li] == "pool" and not last
                    eng = nc.gpsimd if pool_l else nc.vector
                    eng.tensor_tensor(
                        out=w3[:, :w, :], in0=w3[:, :w, :],
                        in1=w3[:, w:2 * w, :], op=OP.add)
                eng = nc.gpsimd if TREE_ENG[3] == "pool" and not last \
                    else nc.vector
                eng.tensor_tensor(
                    out=w3[:, 0:1, :],
                    in0=w3[:, 0:1, :], in1=w3[:, 1:2, :], op=OP.add)
                yeng = nc.gpsimd if YS_ENG == "pool" and not last \
                    else nc.vector
                yeng.tensor_scalar_mul(
                    out=ychunk[:, gi * A:(gi + 1) * A], in0=wp[:, :A],
                    scalar1=rzC[:, gi:gi + 1])

                # decode logits: hidden @ dec_w + b2 (y added below)
                sl = pl[:, gi * A:(gi + 1) * A]
                nc.tensor.matmul(out=sl, lhsT=hidT[:, s * P:(s + 1) * P],
                                 rhs=decw_sb, start=True, stop=False)
                nc.tensor.matmul(out=sl, lhsT=ones1[:], rhs=b2_sb,
                                 start=False, stop=True)

            def emit_tail(ctx):
                # decode softmax for the chunk
                sup0, csup = ctx["sup0"], ctx["csup"]
                wa = csup * A
                eli = sml.tile([P, CSUP * A], f32, tag="eli")
                nc.vector.tensor_tensor(out=eli[:, :wa], in0=ctx["pl"],
                                        in1=ctx["y"][:, :wa], op=OP.add)
                el = sml.tile([P, CSUP * A], bf16, tag="el")
                nc.scalar.activation(out=el[:, :wa], in_=eli[:, :wa],
                                     func=AF.Exp)
                zl = sml.tile([P, CSUP], f32, tag="zl")
                nc.vector.reduce_sum(
                    out=zl[:, :csup],
                    in_=el[:, :wa].rearrange("p (s a) -> p s a", a=A),
                    axis=AX.X)
                rzl = sml.tile([P, CSUP], f32, tag="rzl")
                nc.vector.reciprocal(out=rzl[:, :csup], in_=zl[:, :csup])
                pr = sml.tile([P, CSUP * A], bf16, tag="pr")
                eng = nc.gpsimd if PR_ENG == "pool" and csup > 1 \
                    else nc.vector
                eng.tensor_tensor(
                    out=pr[:, :wa].rearrange("p (s a) -> p s a", a=A),
                    in0=el[:, :wa].rearrange("p (s a) -> p s a", a=A),
                    in1=rzl[:, :csup, None].to_broadcast([P, csup, A]),
                    op=OP.mult)
                nc.sync.dma_start(
                    out=outp[:, sup0 * A:(sup0 + csup) * A], in_=pr[:, :wa])

            # software-pipelined: edge(s) runs one supertile ahead of the
            # attention/value work so the PE never head-of-line blocks on the
            # single zg psum bank
            prev = None
            for ci, (sup0, csup) in enumerate(CHUNKS):
                ctx = open_chunk(ci, sup0, csup)
                for gi in range(csup):
                    s = sup0 + gi
                    gt = emit_edge(s, ctx)
                    if prev is not None:
                        emit_value(*prev)
                        if prev[0] == prev[1]["sup0"] + prev[1]["csup"] - 1:
                            emit_tail(prev[1])
                    prev = (s, ctx, gt)
            emit_value(*prev)
            emit_tail(prev[1])

    nc.compile()
    return nc


def _fold_weights(enc_w, enc_b, msg_w, msg_b, key_w, key_b,
                  in_proj_w, in_proj_b, out_w, out_b, dec_w, dec_b):
    wq, wk, wv = in_proj_w[:C], in_proj_w[C:2 * C], in_proj_w[2 * C:]
    bq, bv = in_proj_b[:C], in_proj_b[2 * C:]
    Wq_eff = msg_w @ wq.T
    bq_eff = msg_b @ wq.T + bq
    Wk_eff = key_w @ wk.T
    Wv_eff = msg_w @ wv.T
    bv_eff = msg_b @ wv.T + bv
    s = np.float32(1.0 / np.sqrt(np.float32(C)))
    Wq2 = (Wq_eff @ Wk_eff.T) * s
    bq2 = (bq_eff @ Wk_eff.T) * s
    W2 = Wv_eff @ out_w @ dec_w
    b2 = bv_eff @ out_w @ dec_w + out_b @ dec_w + dec_b
    enc_w65 = np.concatenate([enc_w, enc_b[None, :]], axis=0)
    return enc_w65.astype(np.float32), Wq2.astype(np.float32), \
        bq2.astype(np.float32), W2.astype(np.float32), b2.astype(np.float32)


def _prep_in_maps(obs, neighbor_idx, enc_w, enc_b, msg_w, msg_b, key_w,
                  key_b, in_proj_w, in_proj_b, out_w, out_b, dec_w, dec_b):
    import ml_dtypes

    bf = ml_dtypes.bfloat16
    obs = np.asarray(obs, dtype=np.float32)
    idx = np.asarray(neighbor_idx).astype(np.int64)

    enc_w65, Wq2, bq2, W2, b2 = _fold_weights(
        np.asarray(enc_w, np.float32), np.asarray(enc_b, np.float32),
        np.asarray(msg_w, np.float32), np.asarray(msg_b, np.float32),
        np.asarray(key_w, np.float32), np.asarray(key_b, np.float32),
        np.asarray(in_proj_w, np.float32), np.asarray(in_proj_b, np.float32),
        np.asarray(out_w, np.float32), np.asarray(out_b, np.float32),
        np.asarray(dec_w, np.float32), np.asarray(dec_b, np.float32))

    obs_b = obs.astype(bf)          # bf16 copy for the edge path
    ones_col = np.ones((1,), bf)

    in_maps = []
    for c in range(NCORES):
        base = c * SHARD
        obsT_shard = np.zeros((IN_DIM + 1, NS), bf)
        obsT_shard[:IN_DIM, :SHARD] = obs_b[base:base + SHARD].T
        obsT_shard[IN_DIM, :] = ones_col

        # per-edge obs gather: column (s*2048 + k*128 + p) holds
        # obs[idx[s*128+p, k]] (supertile-major, k, then partition)
        sh_idx = np.zeros((NS, K), np.int64)
        sh_idx[:SHARD] = idx[base:base + SHARD]
        idx_r = sh_idx.reshape(NSUP, P, K)                  # [s, p, k]
        col_idx = idx_r.transpose(0, 2, 1).reshape(-1)      # [s, k, p]
        og = obs_b[col_idx]                                 # [NS*K, 64] bf16
        obsgT = np.empty((IN_DIM + 1, NS * K), bf)
        obsgT[:IN_DIM] = og.T
        obsgT[IN_DIM] = ones_col

        wpack = np.zeros((P, 865), bf)
        wpack[:IN_DIM + 1, 0:H] = enc_w65.astype(bf)
        wpack[:, H:2 * H] = Wq2.astype(bf)
        wpack[:, 256:256 + A] = np.asarray(dec_w, np.float32).astype(bf)
        wpack[:, 288:288 + A] = W2.astype(bf)
        wpack[0, 320:320 + A] = b2.astype(bf)
        wpack[:IN_DIM + 1, 352:864] = obsT_shard[:, :512]
        wpack[:, 864] = bq2.astype(bf)
        in_maps.append({
            "obsgT": obsgT, "obsT_shard": obsT_shard, "wpack": wpack,
        })
    return in_maps


def kernel(obs, neighbor_idx, enc_w, enc_b, msg_w, msg_b, key_w, key_b,
           in_proj_w, in_proj_b, out_w, out_b, dec_w, dec_b):
    from concourse import bass_utils

    in_maps = _prep_in_maps(
        obs, neighbor_idx, enc_w, enc_b, msg_w, msg_b, key_w, key_b,
        in_proj_w, in_proj_b, out_w, out_b, dec_w, dec_b)

    if "nc" not in _PROG_CACHE:
        _PROG_CACHE["nc"] = _build_program()
    nc = _PROG_CACHE["nc"]

    trace = bool(globals().get("_TRACE_RUN", False))
    res = bass_utils.run_bass_kernel_spmd(nc, in_maps, list(range(NCORES)),
                                          trace=trace)
    if trace:
        _PROG_CACHE["last_result"] = res

    out = np.empty((N, A), np.float32)
    for c in range(NCORES):
        o = np.asarray(res.results[c]["outp"], dtype=np.float32)
        o = o.reshape(P, NSUP, A).transpose(1, 0, 2)
        out[c * SHARD:(c + 1) * SHARD] = o.reshape(NS, A)[:SHARD]
    return out
